# revision 15
# baseline (speedup 1.0000x reference)
"""Trainium2 Bass kernel for nn_DataExpander (dense_mlp), 8 NeuronCores.

Reference computation (B=512, G=20000, H=1024, E=512, O=2048):
    x_expanded  = lrelu(x @ W_ge.T + b_ge)                    [B, H]
    gene_emb    = lrelu(embedding_x @ W_em.T + b_em)          [G, H]
    weights     = softmax(x, axis=1)                          [B, G]
    weighted    = weights @ gene_emb                          [B, H]
    out         = lrelu(concat(x_expanded, weighted) @ W_c.T + b_c)   [B, O]

Sharding: the three big matmuls all contract over the gene axis (G=20000),
so each core takes a 2500-gene shard (padded to 2560 = 20 k-tiles):
  - partial pre-activation x_expanded.T sums + exp(x.T) + partial softmax
    denominator (phase A),
  - gene_emb rows for its genes (phase B, no comm needed),
  - partial softmax-numerator.T sums (phase C).
Cross-core reduction happens in three AllReduces, ordered so the big ones
start as early as possible and overlap the remaining compute: AR0 (pre.T,
fp16, fires after phase A), AR1 (denominator, fp32, tiny), AR2 (num.T,
fp16, fires after phase C). After the reductions every core applies
bias/lrelu/softmax-normalize to the full [2H, B] activation and computes its
256-row slice of out.T (output-feature tensor parallel), so the combiner
matmul keeps N=512 (float32r full-rate). fp16 payloads halve the collective
bytes; the partial magnitudes (<~1e4) are far inside fp16 range and the
2^-11 rounding adds ~1e-3 relative error, well under the fp32 path's needs.

All matmuls run as float32r (reduced-precision fp32 multiply, 4x faster than
plain fp32 on the PE, ~1e-4 relative error), with fp32 PSUM accumulation.

softmax is computed without the max-shift: inputs are N(0,1) so exp() spans
[e^-6, e^6] — no overflow risk in fp32, and softmax is shift-invariant.

The walrus build in this container rejects instructions carrying more than
one sync-wait command, while TileContext emits multi-waits wherever deps
converge; _hoist_multi_waits rewrites those into single-wait engine nops.
"""
import sys

if '/opt/trn_rl_repo' not in sys.path:
    sys.path.insert(0, '/opt/trn_rl_repo')

import numpy as np

import concourse.bass as bass
import concourse.mybir as mybir
import concourse.tile as tile

N_CORES = 8
B = 512          # batch
G = 20000        # genes
GS = G // N_CORES            # 2500 genes per core
KT = 20                      # gene k-tiles per core
GP = KT * 128                # 2560, padded gene shard
H = 1024         # hidden
E = 512          # embed
O = 2048         # output
OS = O // N_CORES            # 256 output rows per core

F32 = mybir.dt.float32
F32R = mybir.dt.float32r
F16 = mybir.dt.float16
AF = mybir.ActivationFunctionType

_CACHE = {}


def _make_nop(nc, engine):
    bb = nc.main_func.blocks[-1]
    n_before = len(bb.instructions)
    nc.engines[engine].nop(nofuse=True)
    assert len(bb.instructions) == n_before + 1
    ins = bb.instructions[-1]
    bb.instructions = bb.instructions[:-1]
    return ins


def _hoist_multi_waits(nc, max_waits=1):
    total = 0
    for f in nc.m.functions:
        for bb in f.blocks:
            out = []
            changed = False
            for ins in bb.instructions:
                si = ins.sync_info
                if si is not None and len(si.on_wait) > max_waits:
                    waits = list(si.on_wait)
                    n_hoist = len(waits) - max_waits
                    for w in waits[:n_hoist]:
                        nop = _make_nop(nc, ins.engine)
                        nop.sync_info = mybir.SyncInfo(on_wait=[w], on_update=[])
                        out.append(nop)
                    ins.sync_info = mybir.SyncInfo(
                        on_wait=waits[n_hoist:], on_update=list(si.on_update)
                    )
                    changed = True
                    total += n_hoist
                out.append(ins)
            if changed:
                bb.instructions = out
    return total


LATE_COLLECTIVES = False


def _build_nc(variant="full", reps=1):
    PAYDT = F32 if variant == "fullf32" else F16
    if variant == "fullf32":
        variant = "full"
    core_ids = list(range(N_CORES))
    nc = bass.Bass(target_bir_lowering=True)

    # [k-tile, 128 genes, 0:1024 = W_ge.T slice | 1024:1536 = x.T slice]
    geblk = nc.declare_dram_parameter("geblk", [KT, 128, H + B], F32, isOutput=False)
    embT = nc.declare_dram_parameter("embT", [E, GP], F32, isOutput=False)
    WemT = nc.declare_dram_parameter("WemT", [E, H], F32, isOutput=False)
    bemb = nc.declare_dram_parameter("bemb", [128, H], F32, isOutput=False)
    ones = nc.declare_dram_parameter("ones", [128, 1], F32, isOutput=False)
    ones1 = nc.declare_dram_parameter("ones1", [1, 128], F32, isOutput=False)
    bge = nc.declare_dram_parameter("bge", [H // 128, 128, 1], F32, isOutput=False)
    WcT = nc.declare_dram_parameter("WcT", [O // 128, 128, OS], F32, isOutput=False)
    bcc = nc.declare_dram_parameter("bcc", [OS // 128, 128, 1], F32, isOutput=False)
    outT = nc.declare_dram_parameter("outT", [OS, B], F32, isOutput=True)

    # embT viewed as [p, k, g, j]: element (128k+p, 128g+j) — lets one DMA
    # fetch the whole [512, 128] gene-column block as an SBUF [128, 4*128].
    embT_v = embT[:].rearrange("(k p) (g j) -> p k g j", p=128, j=128)

    with tile.TileContext(nc) as tc:
        with (
            tc.tile_pool(name="const", bufs=1) as const,
            tc.tile_pool(name="psum", bufs=8, space="PSUM") as psum,
            tc.tile_pool(name="dram", bufs=2, space="DRAM") as dram,
        ):
            # ---- constants ----
            wem_t = [const.tile([128, H], F32R, tag=f"wem{k}", name=f"wem{k}") for k in range(4)]
            for k in range(4):
                nc.sync.dma_start(out=wem_t[k][:], in_=WemT[bass.ts(k, 128), :].bitcast(F32R))
            bemb_t = const.tile([128, H], F32, tag="bemb")
            nc.sync.dma_start(out=bemb_t[:], in_=bemb[:])
            ones_t = const.tile([128, 1], F32R, tag="ones")
            nc.sync.dma_start(out=ones_t[:], in_=ones[:].bitcast(F32R))
            ones1_t = const.tile([1, 128], F32, tag="ones1")
            nc.sync.dma_start(out=ones1_t[:], in_=ones1[:])
            bge_t = [const.tile([128, 1], F32, tag=f"bge{m}", name=f"bge{m}") for m in range(8)]
            for m in range(8):
                nc.sync.dma_start(out=bge_t[m][:], in_=bge[m])
            bcc_t = [const.tile([128, 1], F32, tag=f"bcc{m}", name=f"bcc{m}") for m in range(2)]
            for m in range(2):
                nc.sync.dma_start(out=bcc_t[m][:], in_=bcc[m])

            for _rep in range(reps):
              b_pre = dram.tile([H, B], PAYDT, tag="bpre", name=f"bpre{_rep}")
              b_num = dram.tile([H, B], PAYDT, tag="bnum", name=f"bnum{_rep}")
              b_den = dram.tile([1, B], F32, tag="bden", name=f"bden{_rep}")
              b_pre_o = dram.tile([H, B], PAYDT, addr_space="Shared", tag="bpreo", name=f"bpreo{_rep}")
              b_num_o = dram.tile([H, B], PAYDT, addr_space="Shared", tag="bnumo", name=f"bnumo{_rep}")
              b_den_o = dram.tile([1, B], F32, addr_space="Shared", tag="bdeno", name=f"bdeno{_rep}")
              wc_cm = tc.tile_pool(name="wc", bufs=16)
              wc_pool = wc_cm.__enter__()
              wc_t = []
              for k in range(16):
                  w = wc_pool.tile([128, OS], F32R, tag="wc", name=f"wc{k}")
                  nc.gpsimd.dma_start(out=w[:], in_=WcT[k].bitcast(F32R))
                  wc_t.append(w)
              with (
                  tc.tile_pool(name="gemb", bufs=1) as gemb_p,
                  tc.tile_pool(name="expp", bufs=1) as expp_p,
                  tc.tile_pool(name="embc", bufs=3) as embc_p,
                  tc.tile_pool(name="blk", bufs=3) as blk_p,
                  tc.tile_pool(name="stage", bufs=3) as stage_p,
              ):
                  gene_emb = [gemb_p.tile([128, H], F32R, tag=f"ge{g}", name=f"ge{g}") for g in range(KT)]
                  expT = [expp_p.tile([128, B], F32R, tag=f"ex{g}", name=f"ex{g}") for g in range(KT)]

                  # ---- phase A: pre_ge.T partials + exp (geblk streamed once) ----
                  ps_ge = [psum.tile([128, 512], F32, tag="acc", name=f"psge{i}") for i in range(8)]
                  for k in range(KT):
                      blk = blk_p.tile([128, H + B], F32R, tag="blk")
                      nc.scalar.dma_start(out=blk[:], in_=geblk[k].bitcast(F32R))
                      for m in range(8):
                          nc.tensor.matmul(
                              ps_ge[m][:], blk[:, bass.ts(m, 128)], blk[:, H:H + B],
                              start=(k == 0), stop=(k == KT - 1),
                          )
                      nc.scalar.activation(
                          expT[k][:], blk[:, H:H + B].bitcast(F32), AF.Exp,
                      )
                  for m in range(8):
                      st = stage_p.tile([128, 512], PAYDT, tag="stage")
                      nc.vector.tensor_copy(st[:], ps_ge[m][:])
                      nc.sync.dma_start(out=b_pre[bass.ts(m, 128), :], in_=st[:])

                  # ---- denominator partial: ones.T @ exp ----
                  ps_den = psum.tile([1, 512], F32, tag="acc")
                  for k in range(KT):
                      nc.tensor.matmul(
                          ps_den[:], ones_t[:], expT[k][:],
                          start=(k == 0), stop=(k == KT - 1),
                      )
                  st_den = stage_p.tile([1, 512], F32, tag="stden")
                  nc.vector.tensor_copy(st_den[:], ps_den[:])
                  nc.sync.dma_start(out=b_den[:], in_=st_den[:])

                  if variant != "full":
                      b_pre_o, b_num_o, b_den_o = b_pre, b_num, b_den

                  # ---- phase B: gene_emb[g] = lrelu(embT.T @ WemT + b_em) ----
                  for g in range(KT):
                      ch = embc_p.tile([128, 4 * 128], F32R, tag="embc")
                      nc.sync.dma_start(
                          out=ch[:].rearrange("p (k j) -> p k j", j=128),
                          in_=embT_v[:, :, g, :].bitcast(F32R),
                      )
                      for n in range(2):
                          ps = psum.tile([128, 512], F32, tag="acc")
                          for k in range(4):
                              nc.tensor.matmul(
                                  ps[:], ch[:, bass.ts(k, 128)], wem_t[k][:, bass.ts(n, 512)],
                                  start=(k == 0), stop=(k == 3),
                              )
                          st = stage_p.tile([128, 512], F32, tag="stageb")
                          nc.vector.tensor_add(st[:], ps[:], bemb_t[:, bass.ts(n, 512)])
                          nc.scalar.activation(
                              gene_emb[g][:, bass.ts(n, 512)], st[:], AF.Lrelu,
                              bias=0.0, scale=1.0, alpha=0.01,
                          )

                  # ---- AR0/AR1 fire while num (phase C) still computes ----
                  if variant == "full" and not LATE_COLLECTIVES:
                      nc.gpsimd.collective_compute(
                          "AllReduce", mybir.AluOpType.add,
                          replica_groups=[core_ids],
                          ins=[b_pre.opt()], outs=[b_pre_o.opt()],
                      )
                      nc.gpsimd.collective_compute(
                          "AllReduce", mybir.AluOpType.add,
                          replica_groups=[core_ids],
                          ins=[b_den.opt()], outs=[b_den_o.opt()],
                      )

                  # ---- phase C: numerator.T partials: gene_emb.T-slices @ exp ----
                  for m in range(8):
                      ps = psum.tile([128, 512], F32, tag="acc")
                      for g in range(KT):
                          nc.tensor.matmul(
                              ps[:], gene_emb[g][:, bass.ts(m, 128)], expT[g][:],
                              start=(g == 0), stop=(g == KT - 1),
                          )
                      st = stage_p.tile([128, 512], PAYDT, tag="stage")
                      nc.vector.tensor_copy(st[:], ps[:])
                      nc.gpsimd.dma_start(out=b_num[bass.ts(m, 128), :], in_=st[:])

                  if variant == "full" and LATE_COLLECTIVES:
                      nc.gpsimd.collective_compute(
                          "AllReduce", mybir.AluOpType.add,
                          replica_groups=[core_ids],
                          ins=[b_pre.opt()], outs=[b_pre_o.opt()],
                      )
                      nc.gpsimd.collective_compute(
                          "AllReduce", mybir.AluOpType.add,
                          replica_groups=[core_ids],
                          ins=[b_den.opt()], outs=[b_den_o.opt()],
                      )
                  if variant == "full":
                      nc.gpsimd.collective_compute(
                          "AllReduce", mybir.AluOpType.add,
                          replica_groups=[core_ids],
                          ins=[b_num.opt()], outs=[b_num_o.opt()],
                      )

              if variant == "p1":
                  with tc.tile_pool(name="dump", bufs=2) as dump_p:
                      for m in range(OS // 128):
                          dt_ = dump_p.tile([128, B], PAYDT, tag="dt")
                          nc.sync.dma_start(out=dt_[:], in_=b_pre[bass.ts(m, 128), :])
                          ot = dump_p.tile([128, B], F32, tag="ot")
                          nc.vector.tensor_copy(ot[:], dt_[:])
                          nc.sync.dma_start(out=outT[bass.ts(m, 128), :], in_=ot[:])
                  continue

              # ---- phase D: normalize + combiner on this core's out.T rows ----
              with (
                  tc.tile_pool(name="rp", bufs=4) as r_p,
                  tc.tile_pool(name="comb", bufs=1) as comb_p,
                  tc.tile_pool(name="ph3", bufs=2) as ph3_p,
              ):
                  den_sb = ph3_p.tile([1, B], F32, tag="den")
                  recip = ph3_p.tile([1, B], F32, tag="recip")
                  comb = [comb_p.tile([128, B], F32R, tag=f"cb{k}", name=f"cb{k}") for k in range(16)]
                  nc.sync.dma_start(out=den_sb[:], in_=b_den_o[:])
                  nc.vector.reciprocal(recip[:], den_sb[:])
                  ps_bc = psum.tile([128, 512], F32, tag="acc")
                  nc.tensor.matmul(ps_bc[:], ones1_t[:], recip[:], start=True, stop=True)
                  recip_bc = ph3_p.tile([128, B], PAYDT, tag="recipbc")
                  nc.vector.tensor_copy(recip_bc[:], ps_bc[:])
                  for k in range(16):
                      rt = r_p.tile([128, B], PAYDT, tag="rt")
                      src = b_pre_o if k < 8 else b_num_o
                      nc.sync.dma_start(out=rt[:], in_=src[bass.ts(k % 8, 128), :])
                      if k < 8:
                          nc.scalar.activation(
                              comb[k][:], rt[:], AF.Lrelu,
                              bias=bge_t[k][:], scale=1.0, alpha=0.01,
                          )
                      else:
                          nc.vector.tensor_mul(comb[k][:], rt[:], recip_bc[:])

                  for m in range(OS // 128):
                      ps = psum.tile([128, 512], F32, tag="acc")
                      for k in range(16):
                          nc.tensor.matmul(
                              ps[:], wc_t[k][:, bass.ts(m, 128)], comb[k][:],
                              start=(k == 0), stop=(k == 15),
                          )
                      ot = ph3_p.tile([128, B], F32, tag="ot")
                      nc.scalar.activation(
                          ot[:], ps[:], AF.Lrelu,
                          bias=bcc_t[m][:], scale=1.0, alpha=0.01,
                      )
                      nc.sync.dma_start(out=outT[bass.ts(m, 128), :], in_=ot[:])
              wc_cm.__exit__(None, None, None)

    _hoist_multi_waits(nc)
    return nc


def _prep_inputs(x, embedding_x, W_ge, b_ge, W_em, b_em, W_c, b_c):
    """Build per-core input maps (all fp32, hardcoded sharding)."""
    x = np.ascontiguousarray(x, dtype=np.float32)
    xT = x.T  # [G, B] view
    WgeT = np.asarray(W_ge, np.float32).T  # [G, H] view
    bemb_np = np.tile(np.asarray(b_em, np.float32).reshape(1, H), (128, 1))
    WemT_np = np.ascontiguousarray(np.asarray(W_em, np.float32).T)
    ones_np = np.ones((128, 1), np.float32)
    ones1_np = np.ones((1, 128), np.float32)
    bge_np = np.asarray(b_ge, np.float32).reshape(H // 128, 128, 1)
    WcT_full = np.asarray(W_c, np.float32).T  # [2048 features, 2048 out]

    in_maps = []
    for c in range(N_CORES):
        sl = slice(GS * c, GS * (c + 1))
        blk2d = np.zeros((GP, H + B), np.float32)
        blk2d[:GS, :H] = WgeT[sl]
        blk2d[:GS, H:] = xT[sl]
        blk2d[GS:, H:] = -1e4  # exp() underflows to exactly 0 for pad genes
        embT_c = np.zeros((E, GP), np.float32)
        embT_c[:, :GS] = np.asarray(embedding_x, np.float32)[sl].T
        WcT_c = np.ascontiguousarray(
            WcT_full[:, OS * c:OS * (c + 1)]
        ).reshape(O // 128, 128, OS)
        bcc_c = np.asarray(b_c, np.float32)[OS * c:OS * (c + 1)].reshape(OS // 128, 128, 1)
        in_maps.append({
            "geblk": blk2d.reshape(KT, 128, H + B),
            "embT": embT_c,
            "WemT": WemT_np,
            "bemb": bemb_np,
            "ones": ones_np,
            "ones1": ones1_np,
            "bge": bge_np,
            "WcT": WcT_c,
            "bcc": bcc_c,
        })
    return in_maps


def _get_runner(variant="full", reps=1):
    """Build (once) a cached jitted 8-core runner following bass2jax's
    run_bass_via_pjrt shard_map recipe, so repeated calls don't re-trace."""
    key = ("runner", variant, reps)
    if key in _CACHE:
        return _CACHE[key]

    import jax
    from jax.sharding import Mesh, PartitionSpec
    try:
        from jax.experimental.shard_map import shard_map
    except ImportError:
        from jax.shard_map import shard_map
    from concourse import bass2jax

    bass2jax.install_neuronx_cc_hook()
    nc = _build_nc(variant, reps)

    partition_name = (
        nc.partition_id_tensor.name if nc.partition_id_tensor else None
    )
    in_names = []
    out_names = []
    out_avals = []
    zero_outs = []
    for alloc in nc.m.functions[0].allocations:
        if not isinstance(alloc, mybir.MemoryLocationSet):
            continue
        name = alloc.memorylocations[0].name
        if alloc.kind == "ExternalInput":
            if name != partition_name:
                in_names.append(name)
        elif alloc.kind == "ExternalOutput":
            out_names.append(name)
            shape = tuple(alloc.tensor_shape)
            dtype = mybir.dt.np(alloc.dtype)
            out_avals.append(jax.core.ShapedArray(shape, dtype))
            zero_outs.append(np.zeros(shape, dtype))
    n_params = len(in_names)
    all_in_names = in_names + out_names
    if partition_name is not None:
        all_in_names = all_in_names + [partition_name]

    def _body(*args):
        operands = list(args)
        if partition_name is not None:
            operands.append(bass2jax.partition_id_tensor())
        outs = bass2jax._bass_exec_p.bind(
            *operands,
            out_avals=tuple(out_avals),
            in_names=tuple(all_in_names),
            out_names=tuple(out_names),
            lowering_input_output_aliases=(),
            sim_require_finite=True,
            sim_require_nnan=True,
            nc=nc,
        )
        return tuple(outs)

    devices = jax.devices()[:N_CORES]
    mesh = Mesh(np.asarray(devices), ("core",))
    n_outs = len(out_names)
    sharded = jax.jit(
        shard_map(
            _body,
            mesh=mesh,
            in_specs=(PartitionSpec("core"),) * (n_params + n_outs),
            out_specs=(PartitionSpec("core"),) * n_outs,
            check_rep=False,
        ),
        keep_unused=True,
    )
    runner = {
        "fn": sharded,
        "in_names": in_names,
        "out_names": out_names,
        "zero_outs": zero_outs,
        "mesh": mesh,
    }
    _CACHE[key] = runner
    return runner


def _run(in_maps):
    r = _get_runner()
    concat_in = [
        np.concatenate([in_maps[c][name] for c in range(N_CORES)], axis=0)
        for name in r["in_names"]
    ]
    concat_zeros = [
        np.zeros((N_CORES * z.shape[0], *z.shape[1:]), z.dtype)
        for z in r["zero_outs"]
    ]
    out_arrs = r["fn"](*concat_in, *concat_zeros)
    outT_all = np.asarray(out_arrs[0]).reshape(N_CORES, OS, B)
    return outT_all


def kernel(x, embedding_x, W_ge, b_ge, W_em, b_em, W_c, b_c):
    in_maps = _prep_inputs(x, embedding_x, W_ge, b_ge, W_em, b_em, W_c, b_c)
    outT_all = _run(in_maps)
    # outT_all[c] is rows [OS*c : OS*(c+1)] of out.T -> assemble and transpose
    out_T = outT_all.reshape(O, B)
    return np.ascontiguousarray(out_T.T)


# revision 18
# speedup vs baseline: 15.1664x; 15.1664x over previous
"""Trainium2 Bass kernel for nn_DataExpander (dense_mlp), 8 NeuronCores.

Reference computation (B=512, G=20000, H=1024, E=512, O=2048):
    x_expanded  = lrelu(x @ W_ge.T + b_ge)                    [B, H]
    gene_emb    = lrelu(embedding_x @ W_em.T + b_em)          [G, H]
    weights     = softmax(x, axis=1)                          [B, G]
    weighted    = weights @ gene_emb                          [B, H]
    out         = lrelu(concat(x_expanded, weighted) @ W_c.T + b_c)   [B, O]

Sharding: the three big matmuls all contract over the gene axis (G=20000),
so each core takes a 2500-gene shard (padded to 2560 = 20 k-tiles):
  - partial pre-activation x_expanded.T sums + exp(x.T) + partial softmax
    denominator (phase A),
  - gene_emb rows for its genes (phase B, no comm needed),
  - partial softmax-numerator.T sums (phase C).
Cross-core reduction happens in three AllReduces, ordered so the big ones
start as early as possible and overlap the remaining compute: AR0 (pre.T,
fp16, fires after phase A), AR1 (denominator, fp32, tiny), AR2 (num.T,
fp16, fires after phase C). After the reductions every core applies
bias/lrelu/softmax-normalize to the full [2H, B] activation and computes its
256-row slice of out.T (output-feature tensor parallel), so the combiner
matmul keeps N=512 (float32r full-rate). fp16 payloads halve the collective
bytes; the partial magnitudes (<~1e4) are far inside fp16 range and the
2^-11 rounding adds ~1e-3 relative error, well under the fp32 path's needs.

All matmuls run as float32r (reduced-precision fp32 multiply, 4x faster than
plain fp32 on the PE, ~1e-4 relative error), with fp32 PSUM accumulation.

softmax is computed without the max-shift: inputs are N(0,1) so exp() spans
[e^-6, e^6] — no overflow risk in fp32, and softmax is shift-invariant.

The walrus build in this container rejects instructions carrying more than
one sync-wait command, while TileContext emits multi-waits wherever deps
converge; _hoist_multi_waits rewrites those into single-wait engine nops.
"""
import sys

if '/opt/trn_rl_repo' not in sys.path:
    sys.path.insert(0, '/opt/trn_rl_repo')

import numpy as np

import concourse.bass as bass
import concourse.mybir as mybir
import concourse.tile as tile

N_CORES = 8
B = 512          # batch
G = 20000        # genes
GS = G // N_CORES            # 2500 genes per core
KT = 20                      # gene k-tiles per core
GP = KT * 128                # 2560, padded gene shard
H = 1024         # hidden
E = 512          # embed
O = 2048         # output
OS = O // N_CORES            # 256 output rows per core

F32 = mybir.dt.float32
F32R = mybir.dt.float32r
F16 = mybir.dt.float16
AF = mybir.ActivationFunctionType

_CACHE = {}


def _make_nop(nc, engine):
    bb = nc.main_func.blocks[-1]
    n_before = len(bb.instructions)
    nc.engines[engine].nop(nofuse=True)
    assert len(bb.instructions) == n_before + 1
    ins = bb.instructions[-1]
    bb.instructions = bb.instructions[:-1]
    return ins


def _hoist_multi_waits(nc, max_waits=1):
    total = 0
    for f in nc.m.functions:
        for bb in f.blocks:
            out = []
            changed = False
            for ins in bb.instructions:
                si = ins.sync_info
                if si is not None and len(si.on_wait) > max_waits:
                    waits = list(si.on_wait)
                    n_hoist = len(waits) - max_waits
                    for w in waits[:n_hoist]:
                        nop = _make_nop(nc, ins.engine)
                        nop.sync_info = mybir.SyncInfo(on_wait=[w], on_update=[])
                        out.append(nop)
                    ins.sync_info = mybir.SyncInfo(
                        on_wait=waits[n_hoist:], on_update=list(si.on_update)
                    )
                    changed = True
                    total += n_hoist
                out.append(ins)
            if changed:
                bb.instructions = out
    return total


LATE_COLLECTIVES = False


def _build_nc(variant="full", reps=1):
    PAYDT = F32 if variant == "fullf32" else F16
    if variant == "fullf32":
        variant = "full"
    core_ids = list(range(N_CORES))
    nc = bass.Bass(target_bir_lowering=True)

    # [k-tile, 128 genes, 0:1024 = W_ge.T slice | 1024:1536 = x.T slice]
    geblk = nc.declare_dram_parameter("geblk", [KT, 128, H + B], F32, isOutput=False)
    embT = nc.declare_dram_parameter("embT", [E, GP], F32, isOutput=False)
    WemT = nc.declare_dram_parameter("WemT", [E, H], F32, isOutput=False)
    bemb = nc.declare_dram_parameter("bemb", [128, H], F32, isOutput=False)
    ones = nc.declare_dram_parameter("ones", [128, 1], F32, isOutput=False)
    ones1 = nc.declare_dram_parameter("ones1", [1, 128], F32, isOutput=False)
    bge = nc.declare_dram_parameter("bge", [H // 128, 128, 1], F32, isOutput=False)
    WcT = nc.declare_dram_parameter("WcT", [O // 128, 128, OS], F32, isOutput=False)
    bcc = nc.declare_dram_parameter("bcc", [OS // 128, 128, 1], F32, isOutput=False)
    outT = nc.declare_dram_parameter("outT", [OS, B], F32, isOutput=True)

    # embT viewed as [p, k, g, j]: element (128k+p, 128g+j) — lets one DMA
    # fetch the whole [512, 128] gene-column block as an SBUF [128, 4*128].
    embT_v = embT[:].rearrange("(k p) (g j) -> p k g j", p=128, j=128)

    with tile.TileContext(nc) as tc:
        with (
            tc.tile_pool(name="const", bufs=1) as const,
            tc.tile_pool(name="psum", bufs=8, space="PSUM") as psum,
            tc.tile_pool(name="dram", bufs=2, space="DRAM") as dram,
        ):
            # ---- constants ----
            wem_t = [const.tile([128, H], F32R, tag=f"wem{k}", name=f"wem{k}") for k in range(4)]
            for k in range(4):
                nc.sync.dma_start(out=wem_t[k][:], in_=WemT[bass.ts(k, 128), :].bitcast(F32R))
            bemb_t = const.tile([128, H], F32, tag="bemb")
            nc.sync.dma_start(out=bemb_t[:], in_=bemb[:])
            ones_t = const.tile([128, 1], F32R, tag="ones")
            nc.sync.dma_start(out=ones_t[:], in_=ones[:].bitcast(F32R))
            ones1_t = const.tile([1, 128], F32, tag="ones1")
            nc.sync.dma_start(out=ones1_t[:], in_=ones1[:])
            bge_t = [const.tile([128, 1], F32, tag=f"bge{m}", name=f"bge{m}") for m in range(8)]
            for m in range(8):
                nc.sync.dma_start(out=bge_t[m][:], in_=bge[m])
            bcc_t = [const.tile([128, 1], F32, tag=f"bcc{m}", name=f"bcc{m}") for m in range(2)]
            for m in range(2):
                nc.sync.dma_start(out=bcc_t[m][:], in_=bcc[m])

            for _rep in range(reps):
              b_pre = dram.tile([H, B], PAYDT, tag="bpre", name=f"bpre{_rep}")
              b_num = dram.tile([H, B], PAYDT, tag="bnum", name=f"bnum{_rep}")
              b_den = dram.tile([1, B], F32, tag="bden", name=f"bden{_rep}")
              b_pre_o = dram.tile([H, B], PAYDT, addr_space="Shared", tag="bpreo", name=f"bpreo{_rep}")
              b_num_o = dram.tile([H, B], PAYDT, addr_space="Shared", tag="bnumo", name=f"bnumo{_rep}")
              b_den_o = dram.tile([1, B], F32, addr_space="Shared", tag="bdeno", name=f"bdeno{_rep}")
              wc_cm = tc.tile_pool(name="wc", bufs=16)
              wc_pool = wc_cm.__enter__()
              wc_t = []
              for k in range(16):
                  w = wc_pool.tile([128, OS], F32R, tag="wc", name=f"wc{k}")
                  nc.gpsimd.dma_start(out=w[:], in_=WcT[k].bitcast(F32R))
                  wc_t.append(w)
              with (
                  tc.tile_pool(name="gemb", bufs=1) as gemb_p,
                  tc.tile_pool(name="expp", bufs=1) as expp_p,
                  tc.tile_pool(name="embc", bufs=3) as embc_p,
                  tc.tile_pool(name="blk", bufs=3) as blk_p,
                  tc.tile_pool(name="stage", bufs=3) as stage_p,
              ):
                  gene_emb = [gemb_p.tile([128, H], F32R, tag=f"ge{g}", name=f"ge{g}") for g in range(KT)]
                  expT = [expp_p.tile([128, B], F32R, tag=f"ex{g}", name=f"ex{g}") for g in range(KT)]

                  # ---- phase A: pre_ge.T partials + exp (geblk streamed once) ----
                  ps_ge = [psum.tile([128, 512], F32, tag="acc", name=f"psge{i}") for i in range(8)]
                  for k in range(KT):
                      blk = blk_p.tile([128, H + B], F32R, tag="blk")
                      nc.scalar.dma_start(out=blk[:], in_=geblk[k].bitcast(F32R))
                      for m in range(8):
                          nc.tensor.matmul(
                              ps_ge[m][:], blk[:, bass.ts(m, 128)], blk[:, H:H + B],
                              start=(k == 0), stop=(k == KT - 1),
                          )
                      nc.scalar.activation(
                          expT[k][:], blk[:, H:H + B].bitcast(F32), AF.Exp,
                      )
                  for m in range(8):
                      st = stage_p.tile([128, 512], PAYDT, tag="stage")
                      nc.vector.tensor_copy(st[:], ps_ge[m][:])
                      nc.sync.dma_start(out=b_pre[bass.ts(m, 128), :], in_=st[:])

                  # ---- denominator partial: ones.T @ exp ----
                  ps_den = psum.tile([1, 512], F32, tag="acc")
                  for k in range(KT):
                      nc.tensor.matmul(
                          ps_den[:], ones_t[:], expT[k][:],
                          start=(k == 0), stop=(k == KT - 1),
                      )
                  st_den = stage_p.tile([1, 512], F32, tag="stden")
                  nc.vector.tensor_copy(st_den[:], ps_den[:])
                  nc.sync.dma_start(out=b_den[:], in_=st_den[:])

                  if variant != "full":
                      b_pre_o, b_num_o, b_den_o = b_pre, b_num, b_den

                  # ---- phase B: gene_emb[g] = lrelu(embT.T @ WemT + b_em) ----
                  for g in range(KT):
                      ch = embc_p.tile([128, 4 * 128], F32R, tag="embc")
                      nc.sync.dma_start(
                          out=ch[:].rearrange("p (k j) -> p k j", j=128),
                          in_=embT_v[:, :, g, :].bitcast(F32R),
                      )
                      for n in range(2):
                          ps = psum.tile([128, 512], F32, tag="acc")
                          for k in range(4):
                              nc.tensor.matmul(
                                  ps[:], ch[:, bass.ts(k, 128)], wem_t[k][:, bass.ts(n, 512)],
                                  start=(k == 0), stop=(k == 3),
                              )
                          st = stage_p.tile([128, 512], F32, tag="stageb")
                          nc.vector.tensor_add(st[:], ps[:], bemb_t[:, bass.ts(n, 512)])
                          nc.scalar.activation(
                              gene_emb[g][:, bass.ts(n, 512)], st[:], AF.Lrelu,
                              bias=0.0, scale=1.0, alpha=0.01,
                          )

                  # ---- AR0/AR1 fire while num (phase C) still computes ----
                  if variant == "full" and not LATE_COLLECTIVES:
                      nc.gpsimd.collective_compute(
                          "AllReduce", mybir.AluOpType.add,
                          replica_groups=[core_ids],
                          ins=[b_pre.opt()], outs=[b_pre_o.opt()],
                      )
                      nc.gpsimd.collective_compute(
                          "AllReduce", mybir.AluOpType.add,
                          replica_groups=[core_ids],
                          ins=[b_den.opt()], outs=[b_den_o.opt()],
                      )

                  # ---- phase C: numerator.T partials: gene_emb.T-slices @ exp ----
                  for m in range(8):
                      ps = psum.tile([128, 512], F32, tag="acc")
                      for g in range(KT):
                          nc.tensor.matmul(
                              ps[:], gene_emb[g][:, bass.ts(m, 128)], expT[g][:],
                              start=(g == 0), stop=(g == KT - 1),
                          )
                      st = stage_p.tile([128, 512], PAYDT, tag="stage")
                      nc.vector.tensor_copy(st[:], ps[:])
                      nc.gpsimd.dma_start(out=b_num[bass.ts(m, 128), :], in_=st[:])

                  if variant == "full" and LATE_COLLECTIVES:
                      nc.gpsimd.collective_compute(
                          "AllReduce", mybir.AluOpType.add,
                          replica_groups=[core_ids],
                          ins=[b_pre.opt()], outs=[b_pre_o.opt()],
                      )
                      nc.gpsimd.collective_compute(
                          "AllReduce", mybir.AluOpType.add,
                          replica_groups=[core_ids],
                          ins=[b_den.opt()], outs=[b_den_o.opt()],
                      )
                  if variant == "full":
                      nc.gpsimd.collective_compute(
                          "AllReduce", mybir.AluOpType.add,
                          replica_groups=[core_ids],
                          ins=[b_num.opt()], outs=[b_num_o.opt()],
                      )

              if variant == "p1":
                  with tc.tile_pool(name="dump", bufs=2) as dump_p:
                      for m in range(OS // 128):
                          dt_ = dump_p.tile([128, B], PAYDT, tag="dt")
                          nc.sync.dma_start(out=dt_[:], in_=b_pre[bass.ts(m, 128), :])
                          ot = dump_p.tile([128, B], F32, tag="ot")
                          nc.vector.tensor_copy(ot[:], dt_[:])
                          nc.sync.dma_start(out=outT[bass.ts(m, 128), :], in_=ot[:])
                  continue

              # ---- phase D: normalize + combiner on this core's out.T rows ----
              with (
                  tc.tile_pool(name="rp", bufs=4) as r_p,
                  tc.tile_pool(name="comb", bufs=1) as comb_p,
                  tc.tile_pool(name="ph3", bufs=2) as ph3_p,
              ):
                  den_sb = ph3_p.tile([1, B], F32, tag="den")
                  recip = ph3_p.tile([1, B], F32, tag="recip")
                  comb = [comb_p.tile([128, B], F32R, tag=f"cb{k}", name=f"cb{k}") for k in range(16)]
                  nc.sync.dma_start(out=den_sb[:], in_=b_den_o[:])
                  nc.vector.reciprocal(recip[:], den_sb[:])
                  ps_bc = psum.tile([128, 512], F32, tag="acc")
                  nc.tensor.matmul(ps_bc[:], ones1_t[:], recip[:], start=True, stop=True)
                  recip_bc = ph3_p.tile([128, B], PAYDT, tag="recipbc")
                  nc.vector.tensor_copy(recip_bc[:], ps_bc[:])
                  for k in range(16):
                      rt = r_p.tile([128, B], PAYDT, tag="rt")
                      src = b_pre_o if k < 8 else b_num_o
                      nc.sync.dma_start(out=rt[:], in_=src[bass.ts(k % 8, 128), :])
                      if k < 8:
                          nc.scalar.activation(
                              comb[k][:], rt[:], AF.Lrelu,
                              bias=bge_t[k][:], scale=1.0, alpha=0.01,
                          )
                      else:
                          nc.vector.tensor_mul(comb[k][:], rt[:], recip_bc[:])

                  psd = [psum.tile([128, 512], F32, tag="acc", name=f"psd{m}") for m in range(2)]
                  for k0, k1 in ((0, 8), (8, 16)):
                      for m in range(OS // 128):
                          for k in range(k0, k1):
                              nc.tensor.matmul(
                                  psd[m][:], wc_t[k][:, bass.ts(m, 128)], comb[k][:],
                                  start=(k == 0), stop=(k == 15),
                              )
                  for m in range(OS // 128):
                      ps = psd[m]
                      ot = ph3_p.tile([128, B], F32, tag="ot")
                      nc.scalar.activation(
                          ot[:], ps[:], AF.Lrelu,
                          bias=bcc_t[m][:], scale=1.0, alpha=0.01,
                      )
                      nc.sync.dma_start(out=outT[bass.ts(m, 128), :], in_=ot[:])
              wc_cm.__exit__(None, None, None)

    _hoist_multi_waits(nc)
    return nc


def _prep_inputs(x, embedding_x, W_ge, b_ge, W_em, b_em, W_c, b_c):
    """Build per-core input maps (all fp32, hardcoded sharding)."""
    x = np.ascontiguousarray(x, dtype=np.float32)
    xT = x.T  # [G, B] view
    WgeT = np.asarray(W_ge, np.float32).T  # [G, H] view
    bemb_np = np.tile(np.asarray(b_em, np.float32).reshape(1, H), (128, 1))
    WemT_np = np.ascontiguousarray(np.asarray(W_em, np.float32).T)
    ones_np = np.ones((128, 1), np.float32)
    ones1_np = np.ones((1, 128), np.float32)
    bge_np = np.asarray(b_ge, np.float32).reshape(H // 128, 128, 1)
    WcT_full = np.asarray(W_c, np.float32).T  # [2048 features, 2048 out]

    in_maps = []
    for c in range(N_CORES):
        sl = slice(GS * c, GS * (c + 1))
        blk2d = np.zeros((GP, H + B), np.float32)
        blk2d[:GS, :H] = WgeT[sl]
        blk2d[:GS, H:] = xT[sl]
        blk2d[GS:, H:] = -1e4  # exp() underflows to exactly 0 for pad genes
        embT_c = np.zeros((E, GP), np.float32)
        embT_c[:, :GS] = np.asarray(embedding_x, np.float32)[sl].T
        WcT_c = np.ascontiguousarray(
            WcT_full[:, OS * c:OS * (c + 1)]
        ).reshape(O // 128, 128, OS)
        bcc_c = np.asarray(b_c, np.float32)[OS * c:OS * (c + 1)].reshape(OS // 128, 128, 1)
        in_maps.append({
            "geblk": blk2d.reshape(KT, 128, H + B),
            "embT": embT_c,
            "WemT": WemT_np,
            "bemb": bemb_np,
            "ones": ones_np,
            "ones1": ones1_np,
            "bge": bge_np,
            "WcT": WcT_c,
            "bcc": bcc_c,
        })
    return in_maps


def _get_runner(variant="full", reps=1):
    """Build (once) a cached jitted 8-core runner following bass2jax's
    run_bass_via_pjrt shard_map recipe, so repeated calls don't re-trace."""
    key = ("runner", variant, reps)
    if key in _CACHE:
        return _CACHE[key]

    import jax
    from jax.sharding import Mesh, PartitionSpec
    try:
        from jax.experimental.shard_map import shard_map
    except ImportError:
        from jax.shard_map import shard_map
    from concourse import bass2jax

    bass2jax.install_neuronx_cc_hook()
    nc = _build_nc(variant, reps)

    partition_name = (
        nc.partition_id_tensor.name if nc.partition_id_tensor else None
    )
    in_names = []
    out_names = []
    out_avals = []
    zero_outs = []
    for alloc in nc.m.functions[0].allocations:
        if not isinstance(alloc, mybir.MemoryLocationSet):
            continue
        name = alloc.memorylocations[0].name
        if alloc.kind == "ExternalInput":
            if name != partition_name:
                in_names.append(name)
        elif alloc.kind == "ExternalOutput":
            out_names.append(name)
            shape = tuple(alloc.tensor_shape)
            dtype = mybir.dt.np(alloc.dtype)
            out_avals.append(jax.core.ShapedArray(shape, dtype))
            zero_outs.append(np.zeros(shape, dtype))
    n_params = len(in_names)
    all_in_names = in_names + out_names
    if partition_name is not None:
        all_in_names = all_in_names + [partition_name]

    def _body(*args):
        operands = list(args)
        if partition_name is not None:
            operands.append(bass2jax.partition_id_tensor())
        outs = bass2jax._bass_exec_p.bind(
            *operands,
            out_avals=tuple(out_avals),
            in_names=tuple(all_in_names),
            out_names=tuple(out_names),
            lowering_input_output_aliases=(),
            sim_require_finite=True,
            sim_require_nnan=True,
            nc=nc,
        )
        return tuple(outs)

    devices = jax.devices()[:N_CORES]
    mesh = Mesh(np.asarray(devices), ("core",))
    n_outs = len(out_names)
    sharded = jax.jit(
        shard_map(
            _body,
            mesh=mesh,
            in_specs=(PartitionSpec("core"),) * (n_params + n_outs),
            out_specs=(PartitionSpec("core"),) * n_outs,
            check_rep=False,
        ),
        keep_unused=True,
    )
    runner = {
        "fn": sharded,
        "in_names": in_names,
        "out_names": out_names,
        "zero_outs": zero_outs,
        "mesh": mesh,
    }
    _CACHE[key] = runner
    return runner


def _run(in_maps):
    r = _get_runner()
    concat_in = [
        np.concatenate([in_maps[c][name] for c in range(N_CORES)], axis=0)
        for name in r["in_names"]
    ]
    concat_zeros = [
        np.zeros((N_CORES * z.shape[0], *z.shape[1:]), z.dtype)
        for z in r["zero_outs"]
    ]
    out_arrs = r["fn"](*concat_in, *concat_zeros)
    outT_all = np.asarray(out_arrs[0]).reshape(N_CORES, OS, B)
    return outT_all


def kernel(x, embedding_x, W_ge, b_ge, W_em, b_em, W_c, b_c):
    in_maps = _prep_inputs(x, embedding_x, W_ge, b_ge, W_em, b_em, W_c, b_c)
    outT_all = _run(in_maps)
    # outT_all[c] is rows [OS*c : OS*(c+1)] of out.T -> assemble and transpose
    out_T = outT_all.reshape(O, B)
    return np.ascontiguousarray(out_T.T)
